# revision 17
# baseline (speedup 1.0000x reference)
import sys, os
sys.path.insert(0, "/opt/trn_rl_repo")
import numpy as np
import ml_dtypes

from concourse import bass, mybir
from concourse.tile import TileContext
from concourse import bass_utils

f32 = mybir.dt.float32
bf16 = mybir.dt.bfloat16
u8 = mybir.dt.uint8

N_PRE, N_NEU, D, T, NCORES = 32, 128, 15, 500000, 8
TS = T // NCORES            # 62500 per-shard columns
SUP = 2048                  # super-tile width (4 PSUM banks x 2 bufs)
MT = 512                    # matmul free-dim slice
MROWS = 48                  # mask rows shipped to host
XSTAR_FALLBACK = 16.63553237915039

_cache = {}


def _probe_xstar():
    """Minimal f32 x with jax.nn.sigmoid(x) == 1.0 on the CPU backend (the
    backend the oracle runs on: top_k fails to compile on neuron)."""
    try:
        import jax, jax.numpy as jnp
        cpu = jax.devices("cpu")[0]
        with jax.default_device(cpu):
            lo, hi = np.float32(10.0), np.float32(30.0)
            for _ in range(60):
                mid = np.float32((float(lo) + float(hi)) / 2)
                if mid == lo or mid == hi:
                    break
                if float(jax.nn.sigmoid(jnp.asarray(mid, jnp.float32))) == 1.0:
                    hi = mid
                else:
                    lo = mid
            return float(hi)
    except Exception:
        return XSTAR_FALLBACK


def _build_l1(xstar):
    nc = bass.Bass()
    a4 = nc.declare_dram_parameter("a4", [128, TS + 14], f32, isOutput=False)
    wstat = nc.declare_dram_parameter("wstat", [128, 512], f32, isOutput=False)
    bvec = nc.declare_dram_parameter("bvec", [N_NEU, 1], f32, isOutput=False)
    p_out = nc.declare_dram_parameter("p", [N_NEU, TS], f32, isOutput=True)
    m_out = nc.declare_dram_parameter("mask", [MROWS, TS], u8, isOutput=True)

    nsup = (TS + SUP - 1) // SUP
    with (
        nc.sbuf_tensor([128, 512], f32) as wt,
        nc.sbuf_tensor([N_NEU, 1], f32) as bt,
        nc.sbuf_tensor([128, 2 * (SUP + 12)], f32) as atb,
        nc.sbuf_tensor([128, 2 * SUP], f32) as ltb,
        nc.sbuf_tensor([128, 2 * SUP], f32) as ptb,
        nc.sbuf_tensor([MROWS, 2 * SUP], u8) as mtb,
        nc.psum_tensor([128, 2 * SUP], f32) as psb,
        nc.semaphore() as dma_in,
        nc.semaphore() as dma_po,
        nc.semaphore() as pe_done,
        nc.semaphore() as dve_done,
        nc.semaphore() as act_done,
        nc.Block() as block,
    ):
        def AT(b, sl, n):
            return atb[:, b * (SUP + 12) + sl : b * (SUP + 12) + sl + n]
        def LT(b, sl, n, p0=0, p1=128):
            return ltb[p0:p1, b * SUP + sl : b * SUP + sl + n]
        def PT(b, n):
            return ptb[:, b * SUP : b * SUP + n]
        def MTT(b, n):
            return mtb[:, b * SUP : b * SUP + n]
        def PS(b, sl, n):
            return psb[:, b * SUP + sl : b * SUP + sl + n]

        @block.sync
        def _(sync):
            sync.dma_start(wt[:], wstat[:]).then_inc(dma_in, 16)
            sync.dma_start(bt[:], bvec[:]).then_inc(dma_in, 16)
            for it in range(nsup):
                b = it % 2
                j0 = it * SUP
                w = min(SUP, TS - j0)
                if it >= 2:
                    sync.wait_ge(pe_done, it - 1)
                sync.dma_start(AT(b, 0, w + 12), a4[:, j0 : j0 + w + 12]).then_inc(dma_in, 16)

        @block.tensor
        def _(tensor):
            for it in range(nsup):
                b = it % 2
                w = min(SUP, TS - it * SUP)
                tensor.wait_ge(dma_in, 32 + 16 * (it + 1))
                if it >= 2:
                    tensor.wait_ge(dve_done, 2 * (it - 2) + 1)
                for sl in range(0, w, MT):
                    sw = min(MT, w - sl)
                    for p in range(4):
                        mm = nc.tensor.matmul(
                            PS(b, sl, sw),
                            lhsT=wt[:, p * 128 : (p + 1) * 128],
                            rhs=AT(b, sl + 4 * p, sw),
                            start=(p == 0),
                            stop=(p == 3),
                        )
                        if sl + MT >= w and p == 3:
                            mm.then_inc(pe_done, 1)

        @block.vector
        def _(vector):
            vector.wait_ge(dma_in, 32)
            for it in range(nsup):
                b = it % 2
                w = min(SUP, TS - it * SUP)
                vector.wait_ge(pe_done, it + 1)
                if it >= 2:
                    vector.wait_ge(act_done, it - 1)
                nc.vector.tensor_scalar(
                    LT(b, 0, w), PS(b, 0, w), bt[:, 0:1], None,
                    mybir.AluOpType.add,
                ).then_inc(dve_done, 1)
                if it >= 2:
                    vector.wait_ge(dma_po, 16 * (2 * (it - 2) + 2))
                nc.vector.tensor_scalar(
                    MTT(b, w), LT(b, 0, w, 0, MROWS), float(xstar), None,
                    mybir.AluOpType.is_ge,
                ).then_inc(dve_done, 1)

        @block.scalar
        def _(scalar):
            for it in range(nsup):
                b = it % 2
                w = min(SUP, TS - it * SUP)
                scalar.wait_ge(dve_done, 2 * it + 1)
                if it >= 2:
                    scalar.wait_ge(dma_po, 16 * (2 * (it - 2) + 1))
                nc.scalar.activation(
                    PT(b, w), LT(b, 0, w),
                    mybir.ActivationFunctionType.Sigmoid,
                ).then_inc(act_done, 1)

        @block.gpsimd
        def _(gpsimd):
            for it in range(nsup):
                b = it % 2
                j0 = it * SUP
                w = min(SUP, TS - j0)
                gpsimd.wait_ge(act_done, it + 1)
                gpsimd.dma_start(p_out[:, j0 : j0 + w], PT(b, w)).then_inc(dma_po, 16)
                gpsimd.wait_ge(dve_done, 2 * it + 2)
                gpsimd.dma_start(m_out[:, j0 : j0 + w], MTT(b, w)).then_inc(dma_po, 16)
    return nc


def _build_l2():
    nc = bass.Bass()
    s4 = nc.declare_dram_parameter("s4", [128, TS + 12], u8, isOutput=False)
    wtt = nc.declare_dram_parameter("wtt", [128, 256], bf16, isOutput=False)
    btra = nc.declare_dram_parameter("btra", [N_PRE, 1], f32, isOutput=False)
    e_out = nc.declare_dram_parameter("est", [N_PRE, TS], f32, isOutput=True)

    nsup = (TS + SUP - 1) // SUP
    with (
        nc.sbuf_tensor([128, 256], bf16) as wt,
        nc.sbuf_tensor([N_PRE, 1], f32) as bt,
        nc.sbuf_tensor([128, 2 * (SUP + 12)], u8) as sub,
        nc.sbuf_tensor([128, 2 * (SUP + 12)], bf16) as sfb,
        nc.sbuf_tensor([N_PRE, 2 * SUP], f32) as etb,
        nc.psum_tensor([N_PRE, 2 * SUP], f32) as psb,
        nc.semaphore() as dma_in,
        nc.semaphore() as dma_eo,
        nc.semaphore() as pe_done,
        nc.semaphore() as dve_done,
        nc.Block() as block,
    ):
        def SU(b, n):
            return sub[:, b * (SUP + 12) : b * (SUP + 12) + n]
        def SF(b, sl, n):
            return sfb[:, b * (SUP + 12) + sl : b * (SUP + 12) + sl + n]
        def ET(b, n):
            return etb[:, b * SUP : b * SUP + n]
        def PS(b, sl, n):
            return psb[:, b * SUP + sl : b * SUP + sl + n]

        @block.sync
        def _(sync):
            sync.dma_start(wt[:], wtt[:]).then_inc(dma_in, 16)
            sync.dma_start(bt[:], btra[:]).then_inc(dma_in, 16)
            for it in range(nsup):
                b = it % 2
                j0 = it * SUP
                w = min(SUP, TS - j0)
                if it >= 2:
                    sync.wait_ge(dve_done, 2 * (it - 2) + 1)
                sync.dma_start(SU(b, w + 12), s4[:, j0 : j0 + w + 12]).then_inc(dma_in, 16)

        @block.vector
        def _(vector):
            vector.wait_ge(dma_in, 32)
            for it in range(nsup):
                b = it % 2
                w = min(SUP, TS - it * SUP)
                vector.wait_ge(dma_in, 32 + 16 * (it + 1))
                if it >= 2:
                    vector.wait_ge(pe_done, it - 1)
                nc.vector.tensor_copy(SF(b, 0, w + 12), SU(b, w + 12)).then_inc(dve_done, 1)
                vector.wait_ge(pe_done, it + 1)
                if it >= 2:
                    vector.wait_ge(dma_eo, 16 * (it - 1))
                nc.vector.tensor_scalar(
                    ET(b, w), PS(b, 0, w), bt[:, 0:1], None,
                    mybir.AluOpType.add,
                ).then_inc(dve_done, 1)

        @block.tensor
        def _(tensor):
            for it in range(nsup):
                b = it % 2
                w = min(SUP, TS - it * SUP)
                tensor.wait_ge(dve_done, 2 * it + 1)
                for sl in range(0, w, MT):
                    sw = min(MT, w - sl)
                    idx = 0
                    for term in range(2):
                        for p in range(4):
                            mm = nc.tensor.matmul(
                                PS(b, sl, sw),
                                lhsT=wt[:, term * 128 + p * 32 : term * 128 + (p + 1) * 32],
                                rhs=SF(b, sl + 12 - 4 * p, sw),
                                start=(idx == 0),
                                stop=(idx == 7),
                            )
                            if sl + MT >= w and idx == 7:
                                mm.then_inc(pe_done, 1)
                            idx += 1

        @block.gpsimd
        def _(gpsimd):
            for it in range(nsup):
                b = it % 2
                j0 = it * SUP
                w = min(SUP, TS - j0)
                gpsimd.wait_ge(dve_done, 2 * it + 2)
                gpsimd.dma_start(e_out[:, j0 : j0 + w], ET(b, w)).then_inc(dma_eo, 16)
    return nc


def _host_fallback(a, W, b_conv, b_tra, k):
    import jax, jax.numpy as jnp
    cpu = jax.devices("cpu")[0]
    with jax.default_device(cpu):
        logit = jax.lax.conv_general_dilated(
            jnp.asarray(a)[None], jnp.asarray(W), window_strides=(1,),
            padding=[(D, D)], dimension_numbers=("NCH", "OIH", "NCH"))[0] \
            + jnp.asarray(b_conv)[:, None]
        p_b = jax.nn.sigmoid(logit[:, : -(D + 1)])
        _, idx = jax.lax.top_k(p_b.reshape(-1), int(k))
        spikes = jnp.zeros((p_b.size,), p_b.dtype).at[idx].set(1.0).reshape(p_b.shape)
        Wt = jnp.flip(jnp.asarray(W), axis=-1).transpose(1, 0, 2)
        est = jax.lax.conv_general_dilated(
            spikes[None], Wt, window_strides=(1,), padding=[(D - 1, D - 1)],
            dimension_numbers=("NCH", "OIH", "NCH"))[0] + jnp.asarray(b_tra)[:, None]
        est = jnp.roll(est, -D, axis=1)[:, : -(D - 1)]
        return np.asarray(p_b), np.asarray(spikes), np.asarray(est)


def kernel(a, W, b_conv, b_tra, k):
    a = np.ascontiguousarray(a, dtype=np.float32)
    W = np.ascontiguousarray(W, dtype=np.float32)
    b_conv = np.asarray(b_conv, dtype=np.float32)
    b_tra = np.asarray(b_tra, dtype=np.float32)
    k = int(k)

    xstar = _probe_xstar()

    # ---- launch 1: conv + sigmoid -> p; mask = (logit+b >= x*) rows 0:48 ----
    # A4[32i+c, u] = a_pad[c, t0+u+i] with a_pad = 15 left zeros + a (so that
    # pack p's moving slice at [j0+4p, j0+4p+w) yields taps a[., j-15+4p+i]).
    a_pad = np.zeros((N_PRE, T + 18), np.float32)
    a_pad[:, 15 : 15 + T] = a
    wstat = np.zeros((128, 512), np.float32)
    for p in range(4):
        for i in range(4):
            d = 4 * p + i
            if d < D:
                # rows 32i+c, cols 128p+o
                wstat[32 * i : 32 * i + 32, 128 * p : 128 * p + 128] = W[:, :, d].T
    bvec = b_conv.reshape(N_NEU, 1)

    if "l1" not in _cache:
        _cache["l1"] = _build_l1(xstar)
    in_maps1 = []
    for s in range(NCORES):
        t0 = s * TS
        A4 = np.empty((128, TS + 14), np.float32)
        for i in range(4):
            A4[32 * i : 32 * i + 32, :] = a_pad[:, t0 + i : t0 + i + TS + 14]
        in_maps1.append({"a4": A4, "wstat": wstat, "bvec": bvec})
    res1 = bass_utils.run_bass_kernel_spmd(_cache["l1"], in_maps1,
                                           core_ids=list(range(NCORES)))
    p_full = np.concatenate([r["p"] for r in res1.results], axis=1)
    mask = np.concatenate([r["mask"] for r in res1.results], axis=1)

    # ---- host merge: first-k ties in row-major order -> per-row cutoffs ----
    cnt = mask.sum(axis=1, dtype=np.int64)
    cum = np.cumsum(cnt)
    if cum[-1] < k:
        return _host_fallback(a, W, b_conv, b_tra, k)
    nstar = int(np.argmax(cum >= k))
    if nstar >= N_PRE:
        return _host_fallback(a, W, b_conv, b_tra, k)
    r = k - (int(cum[nstar - 1]) if nstar > 0 else 0)
    pos = np.flatnonzero(mask[nstar])
    t_cut = int(pos[r - 1]) + 1 if r > 0 else 0

    spk_cut = np.zeros((N_PRE, T + 18), np.uint8)
    spk_cut[: nstar + 1, 0:T] = mask[: nstar + 1, :]
    spk_cut[nstar, t_cut:T] = 0

    spikes = np.zeros((N_NEU, T), np.float32)
    spikes[:nstar] = mask[:nstar, :].astype(np.float32)
    spikes[nstar, :t_cut] = mask[nstar, :t_cut].astype(np.float32)

    # ---- launch 2: est = convT(spikes) + b_tra ----
    # S4[32i+o, v] = spk_cut[o, t0+v+3-i]; pack p slice [j0+12-4p, ..+w)
    # yields spikes[o, t+15-(4p+i)].
    Whi = W.astype(ml_dtypes.bfloat16)
    Wlo = (W - Whi.astype(np.float32)).astype(ml_dtypes.bfloat16)
    wtt = np.zeros((128, 256), ml_dtypes.bfloat16)
    for term, Wx in enumerate((Whi, Wlo)):
        Wxf = Wx.astype(np.float32)
        for p in range(4):
            for i in range(4):
                d = 4 * p + i
                if d < D:
                    blk = Wxf[0:N_PRE, :, d]          # (o=32, c=32)
                    wtt[32 * i : 32 * i + 32, term * 128 + 32 * p : term * 128 + 32 * p + 32] = \
                        blk.astype(ml_dtypes.bfloat16)
    btra = b_tra.reshape(N_PRE, 1)

    if "l2" not in _cache:
        _cache["l2"] = _build_l2()
    in_maps2 = []
    for s in range(NCORES):
        t0 = s * TS
        S4 = np.empty((128, TS + 12), np.uint8)
        for i in range(4):
            S4[32 * i : 32 * i + 32, :] = spk_cut[:, t0 + 3 - i : t0 + 3 - i + TS + 12]
        in_maps2.append({"s4": S4, "wtt": wtt, "btra": btra})
    res2 = bass_utils.run_bass_kernel_spmd(_cache["l2"], in_maps2,
                                           core_ids=list(range(NCORES)))
    est = np.concatenate([r["est"] for r in res2.results], axis=1)
    # jnp.roll wraparound: last column sees spikes[:, 0] through tap e=0
    est[:, T - 1] = (b_tra + W[:, :, 0].astype(np.float32).T @ spikes[:, 0]).astype(np.float32)

    return p_full, spikes, est


if __name__ == "__main__":
    pass
